# revision 2
# baseline (speedup 1.0000x reference)
"""Trainium2 Bass kernel for nn_CorrectSplineLinear (embedding_lookup regime).

Math: reference computes
    W[o,t,f] = sum_c interp[o,t,c] * E[c,f]        (interp = piecewise-linear in t)
    out[o,b,t] = sum_f x[b,f] * W[o,t,f]
which collapses algebraically to
    Z[b,(o,s)] = sum_f x[b,f] * cvE[(o,s),f]       (cvE = cv @ E, host-precomputed)
    dZ = shifted column-difference of Z
    out[o,b,t] = Z[b,(o,j(t))] + tl(t)*dZ[b,(o,j(t))]
so no [O,I,I] weight is ever materialized and the device contraction is a
single 4-chunk accumulated matmul straight off the input DMA (no y stage).
All device-side I/O is fp16 (the 2e-2 rel-err budget dwarfs fp16's ~7e-4),
which halves HBM traffic: ~4.3 MiB of output stores per core.

The expansion (out = Z + tl*dZ, per-partition scalars Z,dZ) runs as 96
tensor_scalar-class ops (32 rows x 3 spline segments) greedily balanced
across DVE/ACT/GPSIMD by measured per-op cost; rows are padded to 3x172 =
516 columns so every op is uniform, even-width and 4B-aligned; the host
strips the padding.  Stores are pipelined behind the expansion in row
groups, issued from the otherwise-idle SyncE ring, which sprays across
all 16 DMA engines.

Front-latency tricks: tl is generated on-device (iota + immediate-scalar
ops) during the input-DMA shadow; inputs arrive as 4 chunked DMAs split
across both HWDGE rings so Z matmuls start on first-landed chunks; the
Z->SBUF copy is split (head cols first) so the first rows expand while
the tail of Z is still copying.

Sharding: out_features O=256 split across 8 cores (32 rows each); x
replicated; each core gets its cv@E product pre-transposed.
"""

import sys
from contextlib import ExitStack

import numpy as np

try:
    import concourse.bass as bass
except ImportError:  # fresh grading dir: concourse lives in the repo checkout
    sys.path.insert(0, "/opt/trn_rl_repo")
    import concourse.bass as bass

import concourse.bacc as bacc
import concourse.mybir as mybir
import concourse.tile as tile
from concourse.bass_utils import run_bass_kernel_spmd

N_CORES = 8
O, I, K, C, B = 256, 512, 3, 128, 128
OL = O // N_CORES  # 32 output rows per core
NS = K + 1  # 4 control values per output row
NZ = OL * NS  # 128 Z columns per core
F16 = mybir.dt.float16
F32 = mybir.dt.float32

# ---- spline geometry (input-independent, mirrors reference arithmetic) ----
_t = np.linspace(0.0, 1.0, I).astype(np.float32)
_ts = (_t * np.float32(K)).astype(np.float32)
_j = np.clip(np.floor(_ts), 0.0, float(K - 1)).astype(np.int32)
_TL = (_ts - _j.astype(np.float32)).astype(np.float32)  # [I] local coord in segment
_b0 = int(np.searchsorted(_j, 1))  # first t index in segment 1 (171)
_b1 = int(np.searchsorted(_j, 2))  # first t index in segment 2 (341)
_SEG = [(0, _b0), (_b0, _b1), (_b1, I)]  # per-segment [t0,t1) in true coords

SW = 172  # padded segment width (even, 4B-aligned); true segments are <= 172
RS = 3 * SW  # padded row stride (516 cols)

# tl over the padded grid is affine in the global padded index i per span:
# tl = (3/511)*i + bias_j for i in [172j, 172j+172)
_TL_SCALE = float(np.float32(3.0) / np.float32(511.0))
_TL_BIAS = [float(_TL_SCALE * (t0 - SW * sj) - sj) for sj, (t0, _t1) in enumerate(_SEG)]

# ---- packed-input column layout ([128, _TOT] fp16) ----
# chunk k at 256k: [xT_k (128 b cols) | cvET_k (128 (o,s) cols)]
_TOT = 4 * 256  # 1024

HEAD_ROWS = 4  # rows expanded off the head Z copy
_HC = NS * HEAD_ROWS + 1  # head Z cols (need Z[4o+s+1] for dZ)

# store groups (half-open row ranges); the last is small so its HBM
# completion receipt comes quickly after the final expansion op
_STORE_GROUPS = [(0, 1), (1, 2), (2, 4), (4, 7), (7, 10), (10, 14), (14, 18),
                 (18, 22), (22, 25), (25, 28), (28, 30), (30, 31), (31, 32)]

# measured per-op cost (ns) at 172 cols for greedy engine balancing
_COST = {"v": 171.0, "a": 328.0, "g": 420.0}

_cache: dict = {}


def _schedule_ops():
    """Assign each (row, seg) op to an engine, greedy by accumulated cost.

    Head rows go to DVE so the first store groups fire as early as
    possible; ACT starts handicapped by the rest-of-Z copy it also runs.
    """
    load = {"v": 0.0, "a": 280.0, "g": 0.0}
    plan = []  # (o, seg, engine)
    for o in range(OL):
        for sj in range(3):
            if o < 2:
                eng = "v"
                load["v"] += _COST["v"]
            else:
                eng = min(load, key=lambda e: load[e] + _COST[e])
                load[eng] += _COST[eng]
            plan.append((o, sj, eng))
    return plan


def _build_nc():
    nc = bacc.Bacc(
        "TRN2",
        target_bir_lowering=False,
        debug=False,
        num_devices=N_CORES,
        enable_partition_id=False,
        detect_race_conditions=False,
    )
    pk_d = nc.dram_tensor("pk", [128, _TOT], F16, kind="ExternalInput")
    out_d = nc.dram_tensor("out", [B, OL * RS], F16, kind="ExternalOutput")

    with tile.TileContext(nc) as tc, ExitStack() as ctx:
        constp = ctx.enter_context(tc.tile_pool(name="const", bufs=1))
        psump = ctx.enter_context(
            tc.tile_pool(name="psum", bufs=1, space=bass.MemorySpace.PSUM)
        )
        outp = ctx.enter_context(tc.tile_pool(name="outs", bufs=1))

        pk = constp.tile([128, _TOT], F16)
        # chunked input loads split across both HWDGE rings; each chunk's
        # matmul starts as soon as its data lands
        nc.scalar.dma_start(pk[:, 0:256], pk_d[:, 0:256])  # chunk 0
        nc.sync.dma_start(pk[:, 512:768], pk_d[:, 512:768])  # chunk 2
        nc.scalar.dma_start(pk[:, 256:512], pk_d[:, 256:512])  # chunk 1
        nc.sync.dma_start(pk[:, 768:1024], pk_d[:, 768:1024])  # chunk 3

        # tl over the padded 516-col grid, generated on-device during the
        # DMA shadow: one fp16 iota (exact for 0..515) + 3 immediate ops
        tlq = constp.tile([128, RS], F16)
        nc.gpsimd.iota(
            tlq[:],
            [[1, RS]],
            base=0,
            channel_multiplier=0,
            allow_small_or_imprecise_dtypes=True,
        )
        for sj in range(3):
            nc.vector.tensor_scalar(
                tlq[:, sj * SW : (sj + 1) * SW],
                tlq[:, sj * SW : (sj + 1) * SW],
                _TL_SCALE,
                _TL_BIAS[sj],
                mybir.AluOpType.mult,
                mybir.AluOpType.add,
            )

        # Z[b, (o,s)] = sum_f x[b,f] cvE[(o,s),f]: accumulate over 4 chunks
        # of f, ordered by chunk landing time (scalar ring first, then sync)
        z_ps = psump.tile([128, NZ], F32)
        for mi, k in enumerate([0, 2, 1, 3]):
            base = k * 256
            nc.tensor.matmul(
                z_ps[:],
                pk[:, base : base + B],  # lhsT [f_chunk, b]
                pk[:, base + B : base + 256],  # rhs  [f_chunk, (o,s)]
                start=(mi == 0),
                stop=(mi == 3),
            )

        # ztdz[b, c]: Z at cols 0:128, dZ at 128+c = Z[c+1]-Z[c] (col
        # 128+4o+3 is garbage and never read: j(t) <= 2).  Head cols are
        # copied first so rows 0..HEAD_ROWS-1 expand while the rest copies.
        ztdz = constp.tile([128, 2 * NZ], F32)  # TS scalars are fp32

        nc.vector.tensor_copy(ztdz[:, 0:_HC], z_ps[:, 0:_HC])
        nc.vector.tensor_sub(
            ztdz[:, NZ : NZ + _HC - 1],
            ztdz[:, 1:_HC],
            ztdz[:, 0 : _HC - 1],
        )

        def _ztdz_rest():
            nc.scalar.activation(
                ztdz[:, _HC:NZ],
                z_ps[:, _HC:NZ],
                mybir.ActivationFunctionType.Identity,
            )
            nc.vector.tensor_sub(
                ztdz[:, NZ + _HC - 1 : 2 * NZ - 1],
                ztdz[:, _HC:NZ],
                ztdz[:, _HC - 1 : NZ - 1],
            )

        outs = outp.tile([128, OL * RS], F16)

        plan = _schedule_ops()
        by_row = {}
        for o, sj, eng in plan:
            by_row.setdefault(o, []).append((sj, eng))

        did_rest = False
        for g0, g1 in _STORE_GROUPS:
            if g0 >= HEAD_ROWS and not did_rest:
                _ztdz_rest()
                did_rest = True
            for o in range(g0, g1):
                col = o * RS
                zc = NS * o
                for sj, eng in by_row[o]:
                    c0 = col + sj * SW
                    s0 = sj * SW
                    if eng == "a":
                        nc.scalar.activation(
                            outs[:, c0 : c0 + SW],
                            tlq[:, s0 : s0 + SW],
                            mybir.ActivationFunctionType.Identity,
                            bias=ztdz[:, zc + sj : zc + sj + 1],
                            scale=ztdz[:, NZ + zc + sj : NZ + zc + sj + 1],
                        )
                    else:
                        veng = nc.vector if eng == "v" else nc.gpsimd
                        veng.tensor_scalar(
                            outs[:, c0 : c0 + SW],
                            tlq[:, s0 : s0 + SW],
                            ztdz[:, NZ + zc + sj : NZ + zc + sj + 1],
                            ztdz[:, zc + sj : zc + sj + 1],
                            mybir.AluOpType.mult,
                            mybir.AluOpType.add,
                        )
            nc.sync.dma_start(
                out_d[:, g0 * RS : g1 * RS], outs[:, g0 * RS : g1 * RS]
            )

    nc.compile()
    return nc


def _get_nc():
    if "nc" not in _cache:
        _cache["nc"] = _build_nc()
    return _cache["nc"]


def _pack_inputs(x, control_values, expansion_matrix):
    x = np.ascontiguousarray(x, dtype=np.float32)
    cv = np.ascontiguousarray(control_values, dtype=np.float32)
    E = np.ascontiguousarray(expansion_matrix, dtype=np.float32)

    base = np.zeros((128, _TOT), dtype=np.float16)
    for k in range(4):
        base[:, k * 256 : k * 256 + B] = x[:, k * 128 : (k + 1) * 128].T

    in_maps = []
    for core in range(N_CORES):
        m = base.copy()
        slab = cv[core * OL : (core + 1) * OL].reshape(NZ, C)  # [(o,s), c]
        cvE = (slab @ E).astype(np.float16)  # [(o,s), f]
        for k in range(4):
            m[:, k * 256 + B : k * 256 + 256] = cvE[:, k * 128 : (k + 1) * 128].T
        in_maps.append({"pk": m})
    return in_maps


def _run(in_maps, trace=False):
    nc = _get_nc()
    return run_bass_kernel_spmd(
        nc, in_maps, core_ids=list(range(N_CORES)), trace=trace
    )


def _gather(results):
    # per-core [B, OL*RS] fp16 (padded rows) -> [O, B, I] fp32
    full = np.concatenate(
        [r["out"].reshape(B, OL, 3, SW) for r in results], axis=1
    )  # [B, O, 3, SW]
    out = np.empty((O, B, I), dtype=np.float32)
    fullT = full.transpose(1, 0, 2, 3)  # [O, B, 3, SW]
    for sj, (t0, t1) in enumerate(_SEG):
        out[:, :, t0:t1] = fullT[:, :, sj, 0 : t1 - t0]
    return out


def kernel(x, control_points, control_values, expansion_matrix):
    in_maps = _pack_inputs(x, control_values, expansion_matrix)
    res = _run(in_maps, trace=False)
    return _gather(res.results)


def kernel_traced(x, control_points, control_values, expansion_matrix):
    """Same as kernel() but profiles on HW; returns (out, BassKernelResults)."""
    in_maps = _pack_inputs(x, control_values, expansion_matrix)
    res = _run(in_maps, trace=True)
    return _gather(res.results), res


# revision 7
# speedup vs baseline: 1.1131x; 1.1131x over previous
"""Trainium2 Bass kernel for nn_CorrectSplineLinear (embedding_lookup regime).

Math: reference computes
    W[o,t,f] = sum_c interp[o,t,c] * E[c,f]        (interp = piecewise-linear in t)
    out[o,b,t] = sum_f x[b,f] * W[o,t,f]
which collapses algebraically to
    Z[b,(o,s)] = sum_f x[b,f] * cvE[(o,s),f]       (cvE = cv @ E, host-precomputed)
    dZ = shifted column-difference of Z
    out[o,b,t] = Z[b,(o,j(t))] + tl(t)*dZ[b,(o,j(t))]
so no [O,I,I] weight is ever materialized and the device contraction is a
single 4-chunk accumulated matmul straight off the input DMA (no y stage).
All device-side I/O is fp16 (the 2e-2 rel-err budget dwarfs fp16's ~7e-4),
which halves HBM traffic: ~4.3 MiB of output stores per core.

The expansion (out = Z + tl*dZ, per-partition scalars Z,dZ) runs as 96
tensor_scalar-class ops (32 rows x 3 spline segments) greedily balanced
across DVE/ACT/GPSIMD by measured per-op cost; rows are padded to 3x172 =
516 columns so every op is uniform, even-width and 4B-aligned; the host
strips the padding.  Stores are pipelined behind the expansion in row
groups, issued from the otherwise-idle SyncE ring, which sprays across
all 16 DMA engines.

Front-latency tricks: tl is generated on-device (iota + immediate-scalar
ops) during the input-DMA shadow; inputs arrive as 4 chunked DMAs split
across both HWDGE rings so Z matmuls start on first-landed chunks; the
Z->SBUF copy is split (head cols first) so the first rows expand while
the tail of Z is still copying.

Sharding: out_features O=256 split across 8 cores (32 rows each); x
replicated; each core gets its cv@E product pre-transposed.
"""

import sys
from contextlib import ExitStack

import numpy as np

try:
    import concourse.bass as bass
except ImportError:  # fresh grading dir: concourse lives in the repo checkout
    sys.path.insert(0, "/opt/trn_rl_repo")
    import concourse.bass as bass

import concourse.bacc as bacc
import concourse.mybir as mybir
import concourse.tile as tile
from concourse.bass_utils import run_bass_kernel_spmd

N_CORES = 8
O, I, K, C, B = 256, 512, 3, 128, 128
OL = O // N_CORES  # 32 output rows per core
NS = K + 1  # 4 control values per output row
NZ = OL * NS  # 128 Z columns per core
F16 = mybir.dt.float16
F32 = mybir.dt.float32

# ---- spline geometry (input-independent, mirrors reference arithmetic) ----
_t = np.linspace(0.0, 1.0, I).astype(np.float32)
_ts = (_t * np.float32(K)).astype(np.float32)
_j = np.clip(np.floor(_ts), 0.0, float(K - 1)).astype(np.int32)
_TL = (_ts - _j.astype(np.float32)).astype(np.float32)  # [I] local coord in segment
_b0 = int(np.searchsorted(_j, 1))  # first t index in segment 1 (171)
_b1 = int(np.searchsorted(_j, 2))  # first t index in segment 2 (341)
_SEG = [(0, _b0), (_b0, _b1), (_b1, I)]  # per-segment [t0,t1) in true coords

SW = 172  # padded segment width (even, 4B-aligned); true segments are <= 172
RS = 3 * SW  # padded row stride (516 cols)

# tl over the padded grid is affine in the global padded index i per span:
# tl = (3/511)*i + bias_j for i in [172j, 172j+172)
_TL_SCALE = float(np.float32(3.0) / np.float32(511.0))
_TL_BIAS = [float(_TL_SCALE * (t0 - SW * sj) - sj) for sj, (t0, _t1) in enumerate(_SEG)]

# ---- packed-input column layout ----
# SBUF: chunk k at 256k: [xT_k (128 b cols) | cvET_k (128 (o,s) cols)]
# DRAM: chunk k at 272k (row stride 2176 B, non-power-of-2 so the strided
# chunk DMAs spread across HBM channels instead of aliasing onto one)
_TOT = 4 * 256  # 1024 (SBUF)
_DSTRIDE = 272
_DTOT = 4 * _DSTRIDE  # 1088 (DRAM)

HEAD_ROWS = 4  # rows expanded off the head Z copy
_HC = NS * HEAD_ROWS + 1  # head Z cols (need Z[4o+s+1] for dZ)

# store groups (half-open row ranges); the last is small so its HBM
# completion receipt comes quickly after the final expansion op
_STORE_GROUPS = [(0, 1), (1, 2), (2, 4), (4, 7), (7, 10), (10, 14), (14, 18),
                 (18, 22), (22, 25), (25, 28), (28, 30), (30, 31), (31, 32)]

# measured sustained per-op cost (ticks) at 172 cols for greedy balancing:
# per-partition-scalar tensor_scalar runs at DVE 2x (not 4x), ~224 ticks
_COST = {"v": 224.0, "a": 480.0, "g": 512.0}

_cache: dict = {}


def _schedule_ops():
    """Assign each (row, seg) op to an engine, greedy by accumulated cost.

    Head rows go to DVE so the first store groups fire as early as
    possible; ACT starts handicapped by the rest-of-Z copy it also runs.
    """
    load = {"v": 0.0, "a": 280.0, "g": 0.0}
    plan = []  # (o, seg, engine)
    for o in range(OL):
        for sj in range(3):
            if o < 2:
                eng = "v"
                load["v"] += _COST["v"]
            else:
                eng = min(load, key=lambda e: load[e] + _COST[e])
                load[eng] += _COST[eng]
            plan.append((o, sj, eng))
    return plan


def _build_nc():
    nc = bacc.Bacc(
        "TRN2",
        target_bir_lowering=False,
        debug=False,
        num_devices=N_CORES,
        enable_partition_id=False,
        detect_race_conditions=False,
    )
    pk_d = nc.dram_tensor("pk", [128, _DTOT], F16, kind="ExternalInput")
    out_d = nc.dram_tensor("out", [B, OL * RS], F16, kind="ExternalOutput")

    with tile.TileContext(nc) as tc, ExitStack() as ctx:
        constp = ctx.enter_context(tc.tile_pool(name="const", bufs=1))
        psump = ctx.enter_context(
            tc.tile_pool(name="psum", bufs=1, space=bass.MemorySpace.PSUM)
        )
        outp = ctx.enter_context(tc.tile_pool(name="outs", bufs=1))

        pk = constp.tile([128, _TOT], F16)
        # chunked input loads split across both HWDGE rings; each chunk's
        # matmul starts as soon as its data lands
        for ring, k in [(nc.scalar, 0), (nc.sync, 2), (nc.scalar, 1), (nc.sync, 3)]:
            ring.dma_start(
                pk[:, k * 256 : k * 256 + 256],
                pk_d[:, k * _DSTRIDE : k * _DSTRIDE + 256],
            )

        # tl over the padded 516-col grid, generated on-device during the
        # DMA shadow: one fp16 iota (exact for 0..515) + 3 immediate ops
        tlq = constp.tile([128, RS], F16)
        nc.gpsimd.iota(
            tlq[:],
            [[1, RS]],
            base=0,
            channel_multiplier=0,
            allow_small_or_imprecise_dtypes=True,
        )
        for sj in range(3):
            nc.vector.tensor_scalar(
                tlq[:, sj * SW : (sj + 1) * SW],
                tlq[:, sj * SW : (sj + 1) * SW],
                _TL_SCALE,
                _TL_BIAS[sj],
                mybir.AluOpType.mult,
                mybir.AluOpType.add,
            )

        # Z[b, (o,s)] = sum_f x[b,f] cvE[(o,s),f]: accumulate over 4 chunks
        # of f, ordered by chunk landing time (scalar ring first, then sync)
        z_ps = psump.tile([128, NZ], F32)
        for mi, k in enumerate([0, 2, 1, 3]):
            base = k * 256
            nc.tensor.matmul(
                z_ps[:],
                pk[:, base : base + B],  # lhsT [f_chunk, b]
                pk[:, base + B : base + 256],  # rhs  [f_chunk, (o,s)]
                start=(mi == 0),
                stop=(mi == 3),
            )

        # ztdz[b, c]: Z at cols 0:128, dZ at 128+c = Z[c+1]-Z[c] (col
        # 128+4o+3 is garbage and never read: j(t) <= 2).  Head cols are
        # copied first so rows 0..HEAD_ROWS-1 expand while the rest copies.
        ztdz = constp.tile([128, 2 * NZ], F32)  # TS scalars are fp32

        nc.vector.tensor_copy(ztdz[:, 0:_HC], z_ps[:, 0:_HC])
        nc.vector.tensor_sub(
            ztdz[:, NZ : NZ + _HC - 1],
            ztdz[:, 1:_HC],
            ztdz[:, 0 : _HC - 1],
        )

        def _ztdz_rest():
            nc.scalar.activation(
                ztdz[:, _HC:NZ],
                z_ps[:, _HC:NZ],
                mybir.ActivationFunctionType.Identity,
            )
            nc.vector.tensor_sub(
                ztdz[:, NZ + _HC - 1 : 2 * NZ - 1],
                ztdz[:, _HC:NZ],
                ztdz[:, _HC - 1 : NZ - 1],
            )

        outs = outp.tile([128, OL * RS], F16)

        plan = _schedule_ops()
        by_row = {}
        for o, sj, eng in plan:
            by_row.setdefault(o, []).append((sj, eng))

        did_rest = False
        for g0, g1 in _STORE_GROUPS:
            if g0 >= HEAD_ROWS and not did_rest:
                _ztdz_rest()
                did_rest = True
            for o in range(g0, g1):
                col = o * RS
                zc = NS * o
                for sj, eng in by_row[o]:
                    c0 = col + sj * SW
                    s0 = sj * SW
                    if eng == "a":
                        nc.scalar.activation(
                            outs[:, c0 : c0 + SW],
                            tlq[:, s0 : s0 + SW],
                            mybir.ActivationFunctionType.Identity,
                            bias=ztdz[:, zc + sj : zc + sj + 1],
                            scale=ztdz[:, NZ + zc + sj : NZ + zc + sj + 1],
                        )
                    else:
                        veng = nc.vector if eng == "v" else nc.gpsimd
                        veng.tensor_scalar(
                            outs[:, c0 : c0 + SW],
                            tlq[:, s0 : s0 + SW],
                            ztdz[:, NZ + zc + sj : NZ + zc + sj + 1],
                            ztdz[:, zc + sj : zc + sj + 1],
                            mybir.AluOpType.mult,
                            mybir.AluOpType.add,
                        )
            nc.sync.dma_start(
                out_d[:, g0 * RS : g1 * RS], outs[:, g0 * RS : g1 * RS]
            )

    nc.compile()
    return nc


def _get_nc():
    if "nc" not in _cache:
        _cache["nc"] = _build_nc()
    return _cache["nc"]


def _pack_inputs(x, control_values, expansion_matrix):
    x = np.ascontiguousarray(x, dtype=np.float32)
    cv = np.ascontiguousarray(control_values, dtype=np.float32)
    E = np.ascontiguousarray(expansion_matrix, dtype=np.float32)

    base = np.zeros((128, _DTOT), dtype=np.float16)
    for k in range(4):
        base[:, k * _DSTRIDE : k * _DSTRIDE + B] = x[:, k * 128 : (k + 1) * 128].T

    in_maps = []
    for core in range(N_CORES):
        m = base.copy()
        slab = cv[core * OL : (core + 1) * OL].reshape(NZ, C)  # [(o,s), c]
        cvE = (slab @ E).astype(np.float16)  # [(o,s), f]
        for k in range(4):
            m[:, k * _DSTRIDE + B : k * _DSTRIDE + 256] = (
                cvE[:, k * 128 : (k + 1) * 128].T
            )
        in_maps.append({"pk": m})
    return in_maps


def _run(in_maps, trace=False):
    nc = _get_nc()
    return run_bass_kernel_spmd(
        nc, in_maps, core_ids=list(range(N_CORES)), trace=trace
    )


def _gather(results):
    # per-core [B, OL*RS] fp16 (padded rows) -> [O, B, I] fp32
    full = np.concatenate(
        [r["out"].reshape(B, OL, 3, SW) for r in results], axis=1
    )  # [B, O, 3, SW]
    out = np.empty((O, B, I), dtype=np.float32)
    fullT = full.transpose(1, 0, 2, 3)  # [O, B, 3, SW]
    for sj, (t0, t1) in enumerate(_SEG):
        out[:, :, t0:t1] = fullT[:, :, sj, 0 : t1 - t0]
    return out


def kernel(x, control_points, control_values, expansion_matrix):
    in_maps = _pack_inputs(x, control_values, expansion_matrix)
    res = _run(in_maps, trace=False)
    return _gather(res.results)


def kernel_traced(x, control_points, control_values, expansion_matrix):
    """Same as kernel() but profiles on HW; returns (out, BassKernelResults)."""
    in_maps = _pack_inputs(x, control_values, expansion_matrix)
    res = _run(in_maps, trace=True)
    return _gather(res.results), res


# revision 10
# speedup vs baseline: 1.1358x; 1.0203x over previous
"""Trainium2 Bass kernel for nn_CorrectSplineLinear (embedding_lookup regime).

Math: reference computes
    W[o,t,f] = sum_c interp[o,t,c] * E[c,f]        (interp = piecewise-linear in t)
    out[o,b,t] = sum_f x[b,f] * W[o,t,f]
which collapses algebraically to
    Z[b,(o,s)] = sum_f x[b,f] * cvE[(o,s),f]       (cvE = cv @ E, host-precomputed)
    dZ = shifted column-difference of Z
    out[o,b,t] = Z[b,(o,j(t))] + tl(t)*dZ[b,(o,j(t))]
so no [O,I,I] weight is ever materialized and the device contraction is a
single 4-chunk accumulated matmul straight off the input DMA (no y stage).
All device-side I/O is fp16 (the 2e-2 rel-err budget dwarfs fp16's ~7e-4),
which halves HBM traffic: ~4.3 MiB of output stores per core.

The expansion (out = Z + tl*dZ, per-partition scalars Z,dZ) runs as 96
tensor_scalar-class ops (32 rows x 3 spline segments) greedily balanced
across DVE/ACT/GPSIMD by measured per-op cost; rows are padded to 3x172 =
516 columns so every op is uniform, even-width and 4B-aligned; the host
strips the padding.  Stores are pipelined behind the expansion in row
groups, issued from the otherwise-idle SyncE ring, which sprays across
all 16 DMA engines.

Front-latency tricks: tl is generated on-device (iota + immediate-scalar
ops) during the input-DMA shadow; inputs arrive as 4 chunked DMAs split
across both HWDGE rings so Z matmuls start on first-landed chunks; the
Z->SBUF copy is split (head cols first) so the first rows expand while
the tail of Z is still copying.

Sharding: out_features O=256 split across 8 cores (32 rows each); x
replicated; each core gets its cv@E product pre-transposed.
"""

import sys
from contextlib import ExitStack

import numpy as np

try:
    import concourse.bass as bass
except ImportError:  # fresh grading dir: concourse lives in the repo checkout
    sys.path.insert(0, "/opt/trn_rl_repo")
    import concourse.bass as bass

import concourse.bacc as bacc
import concourse.mybir as mybir
import concourse.tile as tile
from concourse.bass_utils import run_bass_kernel_spmd

N_CORES = 8
O, I, K, C, B = 256, 512, 3, 128, 128
OL = O // N_CORES  # 32 output rows per core
NS = K + 1  # 4 control values per output row
NZ = OL * NS  # 128 Z columns per core
F16 = mybir.dt.float16
F32 = mybir.dt.float32

# ---- spline geometry (input-independent, mirrors reference arithmetic) ----
_t = np.linspace(0.0, 1.0, I).astype(np.float32)
_ts = (_t * np.float32(K)).astype(np.float32)
_j = np.clip(np.floor(_ts), 0.0, float(K - 1)).astype(np.int32)
_TL = (_ts - _j.astype(np.float32)).astype(np.float32)  # [I] local coord in segment
_b0 = int(np.searchsorted(_j, 1))  # first t index in segment 1 (171)
_b1 = int(np.searchsorted(_j, 2))  # first t index in segment 2 (341)
_SEG = [(0, _b0), (_b0, _b1), (_b1, I)]  # per-segment [t0,t1) in true coords

SW = 172  # padded segment width (even, 4B-aligned); true segments are <= 172
RS = 3 * SW  # padded row stride (516 cols)

# tl over the padded grid is affine in the global padded index i per span:
# tl = (3/511)*i + bias_j for i in [172j, 172j+172)
_TL_SCALE = float(np.float32(3.0) / np.float32(511.0))
_TL_BIAS = [float(_TL_SCALE * (t0 - SW * sj) - sj) for sj, (t0, _t1) in enumerate(_SEG)]

# ---- packed-input column layout ----
# SBUF: chunk k at 256k: [xT_k (128 b cols) | cvET_k (128 (o,s) cols)]
# DRAM: chunk k at 272k (row stride 2176 B, non-power-of-2 so the strided
# chunk DMAs spread across HBM channels instead of aliasing onto one)
_TOT = 4 * 256  # 1024 (SBUF)
_DSTRIDE = 272
_DTOT = 4 * _DSTRIDE  # 1088 (DRAM)

HEAD_ROWS = 4  # rows expanded off the head Z copy
_HC = NS * HEAD_ROWS + 1  # head Z cols (need Z[4o+s+1] for dZ)

# store groups (half-open row ranges); the last is small so its HBM
# completion receipt comes quickly after the final expansion op
_STORE_GROUPS = [(0, 1), (1, 2), (2, 4), (4, 7), (7, 10), (10, 14), (14, 18),
                 (18, 22), (22, 25), (25, 28), (28, 30), (30, 31), (31, 32)]

# measured sustained per-op cost (ticks) at 172 cols for greedy balancing:
# per-partition-scalar tensor_scalar runs at DVE 2x (not 4x), ~248 ticks
_COST = {"v": 248.0, "a": 463.0, "g": 513.0}

_cache: dict = {}


def _schedule_ops():
    """Assign each (row, seg) op to an engine, greedy by accumulated cost.

    Head rows go to DVE so the first store groups fire as early as
    possible; ACT starts handicapped by the rest-of-Z copy it also runs.
    """
    load = {"v": 0.0, "a": 280.0, "g": 0.0}
    plan = []  # (o, seg, engine)
    for o in range(OL):
        for sj in range(3):
            if o < 2:
                eng = "v"
                load["v"] += _COST["v"]
            else:
                eng = min(load, key=lambda e: load[e] + _COST[e])
                load[eng] += _COST[eng]
            plan.append((o, sj, eng))
    return plan


def _build_nc():
    nc = bacc.Bacc(
        "TRN2",
        target_bir_lowering=False,
        debug=False,
        num_devices=N_CORES,
        enable_partition_id=False,
        detect_race_conditions=False,
    )
    pk_d = nc.dram_tensor("pk", [128, _DTOT], F16, kind="ExternalInput")
    out_d = nc.dram_tensor("out", [B, OL * RS], F16, kind="ExternalOutput")

    with tile.TileContext(nc) as tc, ExitStack() as ctx:
        constp = ctx.enter_context(tc.tile_pool(name="const", bufs=1))
        psump = ctx.enter_context(
            tc.tile_pool(name="psum", bufs=1, space=bass.MemorySpace.PSUM)
        )
        outp = ctx.enter_context(tc.tile_pool(name="outs", bufs=1))

        pk = constp.tile([128, _TOT], F16)
        # chunked input loads split across both HWDGE rings; each chunk's
        # matmul starts as soon as its data lands
        for ring, k in [(nc.scalar, 0), (nc.sync, 2), (nc.scalar, 1), (nc.sync, 3)]:
            ring.dma_start(
                pk[:, k * 256 : k * 256 + 256],
                pk_d[:, k * _DSTRIDE : k * _DSTRIDE + 256],
            )

        # tl over the padded 516-col grid, generated on-device during the
        # DMA shadow: one fp16 iota (exact for 0..515) + 3 immediate ops
        tlq = constp.tile([128, RS], F16)
        nc.gpsimd.iota(
            tlq[:],
            [[1, RS]],
            base=0,
            channel_multiplier=0,
            allow_small_or_imprecise_dtypes=True,
        )
        for sj in range(3):
            nc.vector.tensor_scalar(
                tlq[:, sj * SW : (sj + 1) * SW],
                tlq[:, sj * SW : (sj + 1) * SW],
                _TL_SCALE,
                _TL_BIAS[sj],
                mybir.AluOpType.mult,
                mybir.AluOpType.add,
            )

        # Z[b, (o,s)] = sum_f x[b,f] cvE[(o,s),f]: accumulate over 4 chunks
        # of f, ordered by chunk landing time (scalar ring first, then sync)
        z_ps = psump.tile([128, NZ], F32)
        for mi, k in enumerate([0, 2, 1, 3]):
            base = k * 256
            nc.tensor.matmul(
                z_ps[:],
                pk[:, base : base + B],  # lhsT [f_chunk, b]
                pk[:, base + B : base + 256],  # rhs  [f_chunk, (o,s)]
                start=(mi == 0),
                stop=(mi == 3),
            )

        # ztdz[b, c]: Z at cols 0:128, dZ at 128+c = Z[c+1]-Z[c] (col
        # 128+4o+3 is garbage and never read: j(t) <= 2).  Head cols are
        # copied first so rows 0..HEAD_ROWS-1 expand while the rest copies.
        ztdz = constp.tile([128, 2 * NZ], F32)  # TS scalars must be fp32

        nc.vector.tensor_copy(ztdz[:, 0:_HC], z_ps[:, 0:_HC])
        nc.vector.tensor_sub(
            ztdz[:, NZ : NZ + _HC - 1],
            ztdz[:, 1:_HC],
            ztdz[:, 0 : _HC - 1],
        )

        def _ztdz_rest():
            nc.scalar.activation(
                ztdz[:, _HC:NZ],
                z_ps[:, _HC:NZ],
                mybir.ActivationFunctionType.Identity,
            )
            nc.vector.tensor_sub(
                ztdz[:, NZ + _HC - 1 : 2 * NZ - 1],
                ztdz[:, _HC:NZ],
                ztdz[:, _HC - 1 : NZ - 1],
            )

        outs = outp.tile([128, OL * RS], F16)

        plan = _schedule_ops()
        by_row = {}
        for o, sj, eng in plan:
            by_row.setdefault(o, []).append((sj, eng))

        did_rest = False
        for g0, g1 in _STORE_GROUPS:
            if g0 >= HEAD_ROWS and not did_rest:
                _ztdz_rest()
                did_rest = True
            for o in range(g0, g1):
                col = o * RS
                zc = NS * o
                for sj, eng in by_row[o]:
                    c0 = col + sj * SW
                    s0 = sj * SW
                    if eng == "a":
                        nc.scalar.activation(
                            outs[:, c0 : c0 + SW],
                            tlq[:, s0 : s0 + SW],
                            mybir.ActivationFunctionType.Identity,
                            bias=ztdz[:, zc + sj : zc + sj + 1],
                            scale=ztdz[:, NZ + zc + sj : NZ + zc + sj + 1],
                        )
                    else:
                        veng = nc.vector if eng == "v" else nc.gpsimd
                        veng.tensor_scalar(
                            outs[:, c0 : c0 + SW],
                            tlq[:, s0 : s0 + SW],
                            ztdz[:, NZ + zc + sj : NZ + zc + sj + 1],
                            ztdz[:, zc + sj : zc + sj + 1],
                            mybir.AluOpType.mult,
                            mybir.AluOpType.add,
                        )
            nc.sync.dma_start(
                out_d[:, g0 * RS : g1 * RS], outs[:, g0 * RS : g1 * RS]
            )

    nc.compile()
    return nc


def _get_nc():
    if "nc" not in _cache:
        _cache["nc"] = _build_nc()
    return _cache["nc"]


def _pack_inputs(x, control_values, expansion_matrix):
    x = np.ascontiguousarray(x, dtype=np.float32)
    cv = np.ascontiguousarray(control_values, dtype=np.float32)
    E = np.ascontiguousarray(expansion_matrix, dtype=np.float32)

    base = np.zeros((128, _DTOT), dtype=np.float16)
    for k in range(4):
        base[:, k * _DSTRIDE : k * _DSTRIDE + B] = x[:, k * 128 : (k + 1) * 128].T

    in_maps = []
    for core in range(N_CORES):
        m = base.copy()
        slab = cv[core * OL : (core + 1) * OL].reshape(NZ, C)  # [(o,s), c]
        cvE = (slab @ E).astype(np.float16)  # [(o,s), f]
        for k in range(4):
            m[:, k * _DSTRIDE + B : k * _DSTRIDE + 256] = (
                cvE[:, k * 128 : (k + 1) * 128].T
            )
        in_maps.append({"pk": m})
    return in_maps


def _run(in_maps, trace=False):
    nc = _get_nc()
    return run_bass_kernel_spmd(
        nc, in_maps, core_ids=list(range(N_CORES)), trace=trace
    )


def _gather(results):
    # per-core [B, OL*RS] fp16 (padded rows) -> [O, B, I] fp32
    full = np.concatenate(
        [r["out"].reshape(B, OL, 3, SW) for r in results], axis=1
    )  # [B, O, 3, SW]
    out = np.empty((O, B, I), dtype=np.float32)
    fullT = full.transpose(1, 0, 2, 3)  # [O, B, 3, SW]
    for sj, (t0, t1) in enumerate(_SEG):
        out[:, :, t0:t1] = fullT[:, :, sj, 0 : t1 - t0]
    return out


def kernel(x, control_points, control_values, expansion_matrix):
    in_maps = _pack_inputs(x, control_values, expansion_matrix)
    res = _run(in_maps, trace=False)
    return _gather(res.results)


def kernel_traced(x, control_points, control_values, expansion_matrix):
    """Same as kernel() but profiles on HW; returns (out, BassKernelResults)."""
    in_maps = _pack_inputs(x, control_values, expansion_matrix)
    res = _run(in_maps, trace=True)
    return _gather(res.results), res


# revision 12
# speedup vs baseline: 1.3138x; 1.1568x over previous
"""Trainium2 Bass kernel for nn_CorrectSplineLinear (embedding_lookup regime).

Math: reference computes
    W[o,t,f] = sum_c interp[o,t,c] * E[c,f]        (interp = piecewise-linear in t)
    out[o,b,t] = sum_f x[b,f] * W[o,t,f]
which collapses algebraically to
    Z[b,(o,s)] = sum_f x[b,f] * cvE[(o,s),f]       (cvE = cv @ E, host-precomputed)
    dZ = shifted column-difference of Z
    out[o,b,t] = Z[b,(o,j(t))] + tl(t)*dZ[b,(o,j(t))]
so no [O,I,I] weight is ever materialized and the device contraction is a
single 4-chunk accumulated matmul straight off the input DMA (no y stage).
All device-side I/O is fp16 (the 2e-2 rel-err budget dwarfs fp16's ~7e-4),
which halves HBM traffic: ~4.3 MiB of output stores per core.

The expansion (out = Z + tl*dZ, per-partition scalars Z,dZ) runs as 96
tensor_scalar-class ops (32 rows x 3 spline segments) greedily balanced
across DVE/ACT/GPSIMD by measured per-op cost; rows are padded to 3x172 =
516 columns so every op is uniform, even-width and 4B-aligned; the host
strips the padding.  Stores are pipelined behind the expansion in row
groups, issued from the otherwise-idle SyncE ring, which sprays across
all 16 DMA engines.

Front-latency tricks: tl is generated on-device (iota + immediate-scalar
ops) during the input-DMA shadow; inputs arrive as 4 chunked DMAs split
across both HWDGE rings so Z matmuls start on first-landed chunks; the
Z->SBUF copy is split (head cols first) so the first rows expand while
the tail of Z is still copying.

Sharding: out_features O=256 split across 8 cores (32 rows each); x
replicated; each core gets its cv@E product pre-transposed.
"""

import sys
from contextlib import ExitStack

import numpy as np

try:
    import concourse.bass as bass
except ImportError:  # fresh grading dir: concourse lives in the repo checkout
    sys.path.insert(0, "/opt/trn_rl_repo")
    import concourse.bass as bass

import concourse.bacc as bacc
import concourse.mybir as mybir
import concourse.tile as tile
from concourse.bass_utils import run_bass_kernel_spmd

N_CORES = 8
O, I, K, C, B = 256, 512, 3, 128, 128
OL = O // N_CORES  # 32 output rows per core
NS = K + 1  # 4 control values per output row
NZ = OL * NS  # 128 Z columns per core
F16 = mybir.dt.float16
F32 = mybir.dt.float32

# ---- spline geometry (input-independent, mirrors reference arithmetic) ----
_t = np.linspace(0.0, 1.0, I).astype(np.float32)
_ts = (_t * np.float32(K)).astype(np.float32)
_j = np.clip(np.floor(_ts), 0.0, float(K - 1)).astype(np.int32)
_TL = (_ts - _j.astype(np.float32)).astype(np.float32)  # [I] local coord in segment
_b0 = int(np.searchsorted(_j, 1))  # first t index in segment 1 (171)
_b1 = int(np.searchsorted(_j, 2))  # first t index in segment 2 (341)
_SEG = [(0, _b0), (_b0, _b1), (_b1, I)]  # per-segment [t0,t1) in true coords

SW = 172  # padded segment width (even, 4B-aligned); true segments are <= 172
RS = 3 * SW  # padded row stride (516 cols)

# tl over the padded grid is affine in the global padded index i per span:
# tl = (3/511)*i + bias_j for i in [172j, 172j+172)
_TL_SCALE = float(np.float32(3.0) / np.float32(511.0))
_TL_BIAS = [float(_TL_SCALE * (t0 - SW * sj) - sj) for sj, (t0, _t1) in enumerate(_SEG)]

# ---- packed-input column layout ----
# SBUF: chunk k at 256k: [xT_k (128 b cols) | cvET_k (128 (o,s) cols)]
# DRAM: chunk k at 272k (row stride 2176 B, non-power-of-2 so the strided
# chunk DMAs spread across HBM channels instead of aliasing onto one)
_TOT = 4 * 256  # 1024 (SBUF)
_DSTRIDE = 272
_DTOT = 4 * _DSTRIDE  # 1088 (DRAM)

HEAD_ROWS = 4  # rows expanded off the head Z copy
_HC = NS * HEAD_ROWS + 1  # head Z cols (need Z[4o+s+1] for dZ)

# store groups (half-open row ranges); small at the start to prime the
# DMA pipe, consolidated near the end so the SyncE trigger stream (~590
# ticks each) never delays the final transfer, tiny last group so its
# HBM completion receipt comes quickly after the final expansion op
_STORE_GROUPS = [(0, 1), (1, 2), (2, 4), (4, 7), (7, 10), (10, 13), (13, 16),
                 (16, 19), (19, 23), (23, 27), (27, 31), (31, 32)]

# measured sustained per-op cost (ticks) at 172 cols for greedy balancing:
# per-partition-scalar tensor_scalar runs at DVE 2x (not 4x), ~248 ticks
_COST = {"v": 248.0, "a": 463.0, "g": 513.0}

_cache: dict = {}


def _schedule_ops():
    """Assign each (row, seg) op to an engine, greedy by accumulated cost.

    Head rows go to DVE so the first store groups fire as early as
    possible; ACT starts handicapped by the rest-of-Z copy it also runs.
    """
    load = {"v": 0.0, "a": 280.0, "g": 0.0}
    forced = {  # first rows: DVE+GP in parallel for earliest stores;
        0: "vgv", 1: "vva", 31: "vvv"}  # last row all-DVE for a short tail
    plan = []  # (o, seg, engine)
    for o in range(OL):
        for sj in range(3):
            if o in forced:
                eng = forced[o][sj]
            else:
                eng = min(load, key=lambda e: load[e] + _COST[e])
            load[eng] += _COST[eng]
            plan.append((o, sj, eng))
    return plan


def _build_nc():
    nc = bacc.Bacc(
        "TRN2",
        target_bir_lowering=False,
        debug=False,
        num_devices=N_CORES,
        enable_partition_id=False,
        detect_race_conditions=False,
    )
    pk_d = nc.dram_tensor("pk", [128, _DTOT], F16, kind="ExternalInput")
    out_d = nc.dram_tensor("out", [B, OL * RS], F16, kind="ExternalOutput")

    with tile.TileContext(nc) as tc, ExitStack() as ctx:
        constp = ctx.enter_context(tc.tile_pool(name="const", bufs=1))
        psump = ctx.enter_context(
            tc.tile_pool(name="psum", bufs=1, space=bass.MemorySpace.PSUM)
        )
        outp = ctx.enter_context(tc.tile_pool(name="outs", bufs=1))

        pk = constp.tile([128, _TOT], F16)
        # chunked input loads split across both HWDGE rings; each chunk's
        # matmul starts as soon as its data lands
        for ring, k in [(nc.scalar, 0), (nc.sync, 2), (nc.scalar, 1), (nc.sync, 3)]:
            ring.dma_start(
                pk[:, k * 256 : k * 256 + 256],
                pk_d[:, k * _DSTRIDE : k * _DSTRIDE + 256],
            )

        # tl over the padded 516-col grid, generated on-device during the
        # DMA shadow: one fp16 iota (exact for 0..515) + 3 immediate ops
        tlq = constp.tile([128, RS], F16)
        nc.gpsimd.iota(
            tlq[:],
            [[1, RS]],
            base=0,
            channel_multiplier=0,
            allow_small_or_imprecise_dtypes=True,
        )
        for sj in range(3):
            nc.vector.tensor_scalar(
                tlq[:, sj * SW : (sj + 1) * SW],
                tlq[:, sj * SW : (sj + 1) * SW],
                _TL_SCALE,
                _TL_BIAS[sj],
                mybir.AluOpType.mult,
                mybir.AluOpType.add,
            )

        # Z[b, (o,s)] = sum_f x[b,f] cvE[(o,s),f]: accumulate over 4 chunks
        # of f, ordered by chunk landing time (scalar ring first, then sync)
        z_ps = psump.tile([128, NZ], F32)
        for mi, k in enumerate([0, 2, 1, 3]):
            base = k * 256
            nc.tensor.matmul(
                z_ps[:],
                pk[:, base : base + B],  # lhsT [f_chunk, b]
                pk[:, base + B : base + 256],  # rhs  [f_chunk, (o,s)]
                start=(mi == 0),
                stop=(mi == 3),
            )

        # ztdz[b, c]: Z at cols 0:128, dZ at 128+c = Z[c+1]-Z[c] (col
        # 128+4o+3 is garbage and never read: j(t) <= 2).  Head cols are
        # copied first so rows 0..HEAD_ROWS-1 expand while the rest copies.
        ztdz = constp.tile([128, 2 * NZ], F32)  # TS scalars must be fp32

        nc.vector.tensor_copy(ztdz[:, 0:_HC], z_ps[:, 0:_HC])
        nc.vector.tensor_sub(
            ztdz[:, NZ : NZ + _HC - 1],
            ztdz[:, 1:_HC],
            ztdz[:, 0 : _HC - 1],
        )

        def _ztdz_rest():
            nc.scalar.activation(
                ztdz[:, _HC:NZ],
                z_ps[:, _HC:NZ],
                mybir.ActivationFunctionType.Identity,
            )
            nc.vector.tensor_sub(
                ztdz[:, NZ + _HC - 1 : 2 * NZ - 1],
                ztdz[:, _HC:NZ],
                ztdz[:, _HC - 1 : NZ - 1],
            )

        outs = outp.tile([128, OL * RS], F16)

        plan = _schedule_ops()
        by_row = {}
        for o, sj, eng in plan:
            by_row.setdefault(o, []).append((sj, eng))

        did_rest = False
        for g0, g1 in _STORE_GROUPS:
            if g0 >= HEAD_ROWS and not did_rest:
                _ztdz_rest()
                did_rest = True
            for o in range(g0, g1):
                col = o * RS
                zc = NS * o
                for sj, eng in by_row[o]:
                    c0 = col + sj * SW
                    s0 = sj * SW
                    if eng == "a":
                        nc.scalar.activation(
                            outs[:, c0 : c0 + SW],
                            tlq[:, s0 : s0 + SW],
                            mybir.ActivationFunctionType.Identity,
                            bias=ztdz[:, zc + sj : zc + sj + 1],
                            scale=ztdz[:, NZ + zc + sj : NZ + zc + sj + 1],
                        )
                    else:
                        veng = nc.vector if eng == "v" else nc.gpsimd
                        veng.tensor_scalar(
                            outs[:, c0 : c0 + SW],
                            tlq[:, s0 : s0 + SW],
                            ztdz[:, NZ + zc + sj : NZ + zc + sj + 1],
                            ztdz[:, zc + sj : zc + sj + 1],
                            mybir.AluOpType.mult,
                            mybir.AluOpType.add,
                        )
            nc.sync.dma_start(
                out_d[:, g0 * RS : g1 * RS], outs[:, g0 * RS : g1 * RS]
            )

    nc.compile()
    return nc


def _get_nc():
    if "nc" not in _cache:
        _cache["nc"] = _build_nc()
    return _cache["nc"]


def _pack_inputs(x, control_values, expansion_matrix):
    x = np.ascontiguousarray(x, dtype=np.float32)
    cv = np.ascontiguousarray(control_values, dtype=np.float32)
    E = np.ascontiguousarray(expansion_matrix, dtype=np.float32)

    base = np.zeros((128, _DTOT), dtype=np.float16)
    for k in range(4):
        base[:, k * _DSTRIDE : k * _DSTRIDE + B] = x[:, k * 128 : (k + 1) * 128].T

    in_maps = []
    for core in range(N_CORES):
        m = base.copy()
        slab = cv[core * OL : (core + 1) * OL].reshape(NZ, C)  # [(o,s), c]
        cvE = (slab @ E).astype(np.float16)  # [(o,s), f]
        for k in range(4):
            m[:, k * _DSTRIDE + B : k * _DSTRIDE + 256] = (
                cvE[:, k * 128 : (k + 1) * 128].T
            )
        in_maps.append({"pk": m})
    return in_maps


def _run(in_maps, trace=False):
    nc = _get_nc()
    return run_bass_kernel_spmd(
        nc, in_maps, core_ids=list(range(N_CORES)), trace=trace
    )


def _gather(results):
    # per-core [B, OL*RS] fp16 (padded rows) -> [O, B, I] fp32
    full = np.concatenate(
        [r["out"].reshape(B, OL, 3, SW) for r in results], axis=1
    )  # [B, O, 3, SW]
    out = np.empty((O, B, I), dtype=np.float32)
    fullT = full.transpose(1, 0, 2, 3)  # [O, B, 3, SW]
    for sj, (t0, t1) in enumerate(_SEG):
        out[:, :, t0:t1] = fullT[:, :, sj, 0 : t1 - t0]
    return out


def kernel(x, control_points, control_values, expansion_matrix):
    in_maps = _pack_inputs(x, control_values, expansion_matrix)
    res = _run(in_maps, trace=False)
    return _gather(res.results)


def kernel_traced(x, control_points, control_values, expansion_matrix):
    """Same as kernel() but profiles on HW; returns (out, BassKernelResults)."""
    in_maps = _pack_inputs(x, control_values, expansion_matrix)
    res = _run(in_maps, trace=True)
    return _gather(res.results), res
